# revision 1
# baseline (speedup 1.0000x reference)
"""Trainium2 Bass kernel: batched Ising energies E_b = s_b^T J s_b.

state: [1024, 2048] float32 in {0,1};  J: [2048, 2048] float32.
Returns energies [1024] float32.

Strategy (8 NeuronCores): sharding is 2D, 4 column-blocks of J x 2
batch-halves.  Core (r, c) computes, for its batch half and J block,
partial_rc[b] = sum_{j in cols_r} (spins[b,:] @ J[:, j]) * spins[b, j]
via PE matmuls (contraction over all 2048 rows of J) plus a
multiply+reduce on the vector engine.  The host sums the 4 column-block
partials per batch half - no on-device collectives.

J precision modes:
  "f32r": J streamed as FP32R (fp32 truncated to ~FP22 inside the PE,
          full matmul rate for moving dim >= 256).  state ships as
          uint8 and is expanded to +-1 fp32 spins on the otherwise-idle
          vector engine (PE matmul requires both operands 32-bit).
  "hilo": J = bf16 hi + bf16 lo, two accumulated matmul passes
          (fp32-level accuracy, 2x matmul work)
  "hi":   J as bf16 only (fastest, bf16-level accuracy)

All device inputs are pre-arranged on the host into [128, X] layouts that
are contiguous per SBUF partition, so DMA descriptors are 4-16KB and the
loads run near HBM rate on a single HWDGE ring in exact consumption
order.  A burst of small dummy matmuls on a zeroed tile warms the PE
clock gate (HAM) while the loads are in flight.
"""

import sys

if "/opt/trn_rl_repo" not in sys.path:
    sys.path.insert(0, "/opt/trn_rl_repo")

import numpy as np
import ml_dtypes

B, N = 1024, 2048
R, C = 4, 2          # J column-block split x batch split (R*C = 8 cores)
CB = N // R          # 512 J-columns per core
BH = B // C          # 512 samples per core
P = 128
KT = N // P          # 16 contraction tiles
BT = BH // P         # 4 output-partition tiles
CHUNK = 4            # k-tiles per input DMA
N_WARM = 150         # small dummy matmuls to warm the PE clock gate
WARM_N = 64          # free dim of each warmup matmul
K_TAIL = 4           # k-tiles computed per-b at the end (epilogue stagger)
MODE = "f32r"        # "f32r" | "hilo" | "hi"

_cache = {}


def _build_program():
    import concourse.bacc as bacc
    import concourse.mybir as mybir
    import concourse.tile as tile

    bf16 = mybir.dt.bfloat16
    f32 = mybir.dt.float32
    f32r = mybir.dt.float32r
    u8 = mybir.dt.uint8
    jdt = f32r if MODE == "f32r" else bf16

    nc = bacc.Bacc("TRN2", target_bir_lowering=False, debug=False, num_devices=R * C)

    if MODE == "f32r":
        su_ext = nc.dram_tensor("su", [P, KT * BH], u8, kind="ExternalInput").ap()
    else:
        st_ext = nc.dram_tensor("st", [P, KT * BH], bf16, kind="ExternalInput").ap()
    jhi_ext = nc.dram_tensor("jhi", [P, KT * CB], jdt, kind="ExternalInput").ap()
    jlo_ext = (
        nc.dram_tensor("jlo", [P, KT * CB], bf16, kind="ExternalInput").ap()
        if MODE == "hilo"
        else None
    )
    sb_ext = nc.dram_tensor("sb", [P, BT * CB], bf16, kind="ExternalInput").ap()
    out_ext = nc.dram_tensor("part", [BH], f32, kind="ExternalOutput").ap()

    with tile.TileContext(nc) as tc:
        with (
            tc.tile_pool(name="persist", bufs=1) as persist,
            tc.tile_pool(name="work", bufs=3) as work,
            tc.tile_pool(name="psum", bufs=1, space="PSUM") as psum_pool,
            tc.tile_pool(name="warmps", bufs=1, space="PSUM") as warm_pool,
        ):
            sdt = f32r if MODE == "f32r" else bf16
            st_t = persist.tile([P, KT, BH], sdt)
            su_t = (
                persist.tile([P, KT, BH], u8, name="su_t")
                if MODE == "f32r"
                else None
            )
            jhi_t = persist.tile([P, KT, CB], jdt)
            jlo_t = (
                persist.tile([P, KT, CB], bf16, name="jlo_t")
                if MODE == "hilo"
                else None
            )
            sb_t = persist.tile([P, BT, CB], bf16)
            red_all = persist.tile([P, BT], f32)
            warm_src = persist.tile([P, CB], bf16)

            # PE warmup: small dummy matmuls on a zeroed tile keep the HAM
            # activity window busy while the real loads stream in.  Small
            # free dim => the last one never delays the first real matmul.
            nc.vector.memset(warm_src[:], 0.0)
            warm_ps = warm_pool.tile([P, WARM_N], f32)
            for _ in range(N_WARM):
                nc.tensor.matmul(
                    warm_ps, lhsT=warm_src[:, :P], rhs=warm_src[:, :WARM_N],
                    start=True, stop=True,
                )

            # Input loads: chunks with 4-16KB per-partition contiguous
            # runs.
            n_chunks = KT // CHUNK
            if MODE == "f32r":

                def jchunk(k0, k1, eng):
                    eng.dma_start(
                        out=jhi_t[:, k0:k1],
                        in_=jhi_ext[:, k0 * CB : k1 * CB],
                    )

                # All loads ride ONE ring (sync: earlier first byte) in
                # exact consumption order, so arrival order is
                # deterministic - splitting across the two HWDGE rings
                # makes the per-ring share flap with queued bytes and
                # starves whichever ring holds the next-needed chunk.
                # Geometric head chunks: each DMA pays a ~3.4us
                # completion-to-consumer latency, so a tiny first chunk
                # lets matmuls start early while later, larger chunks'
                # latencies pipeline behind compute.
                su3 = su_ext.rearrange("p (k b) -> p k b", b=BH)
                nc.sync.dma_start(out=su_t[:, :CHUNK], in_=su3[:, :CHUNK])
                jchunk(0, 1, nc.sync)
                jchunk(1, 2, nc.sync)
                jchunk(2, 3, nc.sync)
                nc.sync.dma_start(out=su_t[:, CHUNK:], in_=su3[:, CHUNK:])
                bounds = [3, 4, 6, 8, 10, 12, 16]
                for ci in range(len(bounds) - 1):
                    jchunk(bounds[ci], bounds[ci + 1], nc.sync)
                # expand uint8 {0,1} -> +-1.0 spins on the vector engine
                for ci in range(n_chunks):
                    kt = slice(ci * CHUNK, (ci + 1) * CHUNK)
                    nc.vector.tensor_scalar(
                        st_t[:, kt],
                        su_t[:, kt],
                        2.0,
                        -1.0,
                        mybir.AluOpType.mult,
                        mybir.AluOpType.add,
                    )
            else:
                for ci in range(n_chunks):
                    kt = slice(ci * CHUNK, (ci + 1) * CHUNK)
                    ks = slice(ci * CHUNK * BH, (ci + 1) * CHUNK * BH)
                    kc = slice(ci * CHUNK * CB, (ci + 1) * CHUNK * CB)
                    nc.sync.dma_start(out=st_t[:, kt], in_=st_ext[:, ks])
                    nc.scalar.dma_start(out=jhi_t[:, kt], in_=jhi_ext[:, kc])
            sb_eng = nc.sync
            sb_eng.dma_start(out=sb_t[:], in_=sb_ext.rearrange(
                "p (t c) -> p t c", c=CB))
            if MODE == "hilo":
                for ci in range(n_chunks):
                    kt = slice(ci * CHUNK, (ci + 1) * CHUNK)
                    kc = slice(ci * CHUNK * CB, (ci + 1) * CHUNK * CB)
                    eng = nc.sync if ci % 2 == 0 else nc.scalar
                    eng.dma_start(out=jlo_t[:, kt], in_=jlo_ext[:, kc])

            ps_tiles = [
                psum_pool.tile([P, CB], f32, name=f"ps_{b}") for b in range(BT)
            ]

            def mm(b, k, jt, start, stop):
                nc.tensor.matmul(
                    ps_tiles[b],
                    lhsT=st_t[:, k, b * P : (b + 1) * P],
                    rhs=jt[:, k],
                    start=start,
                    stop=stop,
                )

            out3 = out_ext.rearrange("(t p) -> p t", p=P)

            def epilogue(b):
                m = work.tile([P, CB], f32, name="m_epi")
                nc.vector.scalar_tensor_tensor(
                    m[:],
                    ps_tiles[b][:],
                    1.0,
                    sb_t[:, b],
                    mybir.AluOpType.mult,
                    mybir.AluOpType.mult,
                    accum_out=red_all[:, b : b + 1],
                )
                # per-b output DMA: the first three hide under the
                # remaining matmuls, only the last is exposed
                nc.sync.dma_start(out=out3[:, b : b + 1], in_=red_all[:, b : b + 1])

            if MODE == "hilo":
                # hi pass k-outer (4 matmuls runnable per arriving chunk),
                # then lo pass b-outer so epilogues overlap remaining MMs
                for k in range(KT):
                    for b in range(BT):
                        mm(b, k, jhi_t, start=(k == 0), stop=False)
                for b in range(BT):
                    for k in range(KT):
                        mm(b, k, jlo_t, start=False, stop=(k == KT - 1))
                    epilogue(b)
            else:
                # single pass: k-outer for the bulk, the last K_TAIL
                # k-tiles go b-by-b so epilogues overlap the tail matmuls
                for k in range(KT - K_TAIL):
                    for b in range(BT):
                        mm(b, k, jhi_t, start=(k == 0), stop=False)
                for b in range(BT):
                    for k in range(KT - K_TAIL, KT):
                        mm(b, k, jhi_t, start=False, stop=(k == KT - 1))
                    epilogue(b)


    nc.compile()
    return nc


def _part_layout(a, inner):
    """[KT*P, inner] row-major -> [P, KT*inner] contiguous per partition."""
    k = a.shape[0] // P
    return np.ascontiguousarray(
        a.reshape(k, P, inner).transpose(1, 0, 2).reshape(P, k * inner)
    )


def _make_in_maps(state, J):
    bf16 = ml_dtypes.bfloat16
    state = np.asarray(state, dtype=np.float32)
    J = np.asarray(J, dtype=np.float32)

    spins = state * 2.0 - 1.0                       # exact in fp32
    sp_bf = spins.astype(bf16)                      # [B, N], exact (+-1)
    if MODE == "f32r":
        su_all = state.astype(np.uint8).T           # [N, B] {0,1}
        Jhi = J
    else:
        st_all = sp_bf.T                            # [N, B] view
        Jhi = J.astype(bf16)
        if MODE == "hilo":
            Jlo = (J - Jhi.astype(np.float32)).astype(bf16)

    in_maps = []
    placement = []
    for core in range(R * C):
        r, c = divmod(core, C)
        m = {
            "jhi": _part_layout(Jhi[:, r * CB : (r + 1) * CB], CB),
            "sb": _part_layout(
                sp_bf[c * BH : (c + 1) * BH, r * CB : (r + 1) * CB], CB
            ),
        }
        if MODE == "f32r":
            m["su"] = _part_layout(su_all[:, c * BH : (c + 1) * BH], BH)
        else:
            m["st"] = _part_layout(st_all[:, c * BH : (c + 1) * BH], BH)
        if MODE == "hilo":
            m["jlo"] = _part_layout(Jlo[:, r * CB : (r + 1) * CB], CB)
        in_maps.append(m)
        placement.append((r, c))
    return in_maps, placement


def kernel(state, J):
    from concourse.bass_utils import run_bass_kernel_spmd

    if "nc" not in _cache:
        _cache["nc"] = _build_program()
    nc = _cache["nc"]

    in_maps, placement = _make_in_maps(state, J)
    res = run_bass_kernel_spmd(nc, in_maps, list(range(R * C)))

    out = np.zeros(B, dtype=np.float32)
    for core, (r, c) in enumerate(placement):
        out[c * BH : (c + 1) * BH] += res.results[core]["part"]
    return out



# revision 4
# speedup vs baseline: 1.4660x; 1.4660x over previous
"""Trainium2 Bass kernel: batched Ising energies E_b = s_b^T J s_b.

state: [1024, 2048] float32 in {0,1};  J: [2048, 2048] float32.
Returns energies [1024] float32.

Because s_i^2 = 1, E = s^T J s = s^T A s where A folds the symmetric
part of J into the upper block-triangle (A_ij = J_ij + J_ji for i<j,
A_ii = J_ii, zero below).  That halves both the matmul work and the
J bytes moved; A ships as bf16 (rel err ~2.5e-3, tolerance 2e-2).

Sharding (8 cores): 2 batch halves x 4 column groups.  Core (r, c)
owns column tiles {4j+r : j=0..3} of A for batch half c.  For a single
SPMD program across cores with different triangle supports, slot j
accumulates its ctile over a fixed cap of 4(j+1) contraction tiles
(ragged edge zero-padded on the host) and ktile storage order is
permuted per core so ctile 4j+r's spin rows always sit at position
4j+3 (uniform epilogue addressing).

Per core: A tiles are the stationary operand; spins ship directly as
fp8e4 {-1,+1} (exact, 1 byte) and stream as the moving operand -- the
PE accepts bf16 stationary x fp8 moving, so no on-chip expand at all.
psum[j] = g = A_block^T s.  Epilogue per slot: m = psum[j] * spin rows
(DVE, fp8 operand), then a ones-vector matmul reduces m across
partitions into a [1, 512] psum accumulator; one output DMA.  Input
streams ride both HWDGE rings (spins on sync, A on scalar) in exact
consumption order with small head chunks; dummy matmuls warm the PE
clock gate while the first chunks land.
"""

import sys

if "/opt/trn_rl_repo" not in sys.path:
    sys.path.insert(0, "/opt/trn_rl_repo")

import numpy as np
import ml_dtypes

B, N = 1024, 2048
P = 128
KT = N // P          # 16 contraction/column tiles
R, C = 4, 2          # column groups x batch halves
BH = B // C          # 512 samples per core
S_CAP = [4, 8, 12, 16]   # per-slot ktile caps (uniform across cores)
ORDER = [(s, j) for s in range(KT) for j in range(4) if S_CAP[j] > s]
NTILE = len(ORDER)   # 40 stationary tiles per core
N_WARM = 52          # PE clock-gate warmup matmuls
WARM_N = 64

# chunk boundaries (in ktile positions) for the two input streams
SU_CHUNKS = [(0, 2), (2, 8), (8, 16)]
A_CHUNKS = [(0, 2), (2, 8), (8, 16)]

_cache = {}


def _tiles_before(s):
    return sum(1 for (s_, _) in ORDER if s_ < s)


def _build_program():
    import concourse.bacc as bacc
    import concourse.mybir as mybir
    import concourse.tile as tile

    bf16 = mybir.dt.bfloat16
    f32 = mybir.dt.float32
    fp8 = mybir.dt.float8e4

    nc = bacc.Bacc("TRN2", target_bir_lowering=False, debug=False, num_devices=R * C)

    su_ext = nc.dram_tensor("su", [P, KT * BH], fp8, kind="ExternalInput").ap()
    a_ext = nc.dram_tensor("a", [P, NTILE * P], bf16, kind="ExternalInput").ap()
    out_ext = nc.dram_tensor("part", [1, BH], f32, kind="ExternalOutput").ap()

    with tile.TileContext(nc) as tc:
        with (
            tc.tile_pool(name="persist", bufs=1) as persist,
            tc.tile_pool(name="work", bufs=1) as work,
            tc.tile_pool(name="psum", bufs=1, space="PSUM") as psum_pool,
            tc.tile_pool(name="warmps", bufs=1, space="PSUM") as warm_pool,
        ):
            su_t = persist.tile([P, KT, BH], fp8)
            a_t = persist.tile([P, NTILE, P], bf16)
            ones_t = persist.tile([P, 1], bf16)
            warm_src = persist.tile([P, P], bf16)
            red_sb = persist.tile([1, BH], f32)
            m_t = [work.tile([P, BH], bf16, name=f"m_{j}") for j in range(4)]

            ps = [psum_pool.tile([P, BH], f32, name=f"ps_{j}") for j in range(4)]
            ep = psum_pool.tile([1, BH], f32, name="ep")
            warm_ps = warm_pool.tile([P, WARM_N], f32)

            # constants via gpsimd (released early in the preamble)
            nc.gpsimd.memset(warm_src[:], 0.0)
            nc.gpsimd.memset(ones_t[:], 1.0)

            # PE warmup: dummy matmuls keep the HAM activity window busy
            # while the first input chunks land.
            for _ in range(N_WARM):
                nc.tensor.matmul(
                    warm_ps, lhsT=warm_src[:, :P], rhs=warm_src[:, :WARM_N],
                    start=True, stop=True,
                )

            # input streams: spins on the sync HWDGE ring, A on the
            # scalar ring, in exact consumption order, small head chunks.
            su3 = su_ext.rearrange("p (k b) -> p k b", b=BH)
            a3 = a_ext.rearrange("p (t q) -> p t q", q=P)
            for ci in range(len(SU_CHUNKS)):
                k0, k1 = SU_CHUNKS[ci]
                nc.sync.dma_start(out=su_t[:, k0:k1], in_=su3[:, k0:k1])
                s0, s1 = A_CHUNKS[ci]
                t0, t1 = _tiles_before(s0), _tiles_before(s1)
                nc.scalar.dma_start(out=a_t[:, t0:t1], in_=a3[:, t0:t1])

            def epilogue(j):
                # m = ps[j] * spin rows of ctile j (position 4j+3), then
                # reduce across partitions via a ones-vector matmul.
                nc.vector.scalar_tensor_tensor(
                    m_t[j][:],
                    ps[j][:],
                    1.0,
                    su_t[:, 4 * j + 3],
                    mybir.AluOpType.mult,
                    mybir.AluOpType.mult,
                )
                nc.tensor.matmul(
                    ep,
                    lhsT=ones_t[:],
                    rhs=m_t[j][:],
                    start=(j == 0),
                    stop=(j == 3),
                )

            ti = 0
            for s in range(KT):
                for j in range(4):
                    if S_CAP[j] <= s:
                        continue
                    nc.tensor.matmul(
                        ps[j],
                        lhsT=a_t[:, ti],
                        rhs=su_t[:, s],
                        start=(s == 0),
                        stop=(s == S_CAP[j] - 1),
                    )
                    ti += 1
                for j in range(4):
                    if s == S_CAP[j] - 1:
                        epilogue(j)
            assert ti == NTILE

            # PSUM -> SBUF -> DRAM (DMA cannot read PSUM directly)
            nc.vector.tensor_scalar(
                red_sb[:], ep[:], 1.0, 0.0,
                mybir.AluOpType.mult, mybir.AluOpType.add,
            )
            nc.sync.dma_start(out=out_ext, in_=red_sb[:])

    nc.compile()
    return nc


def _make_in_maps(state, J):
    bf16 = ml_dtypes.bfloat16
    fp8 = ml_dtypes.float8_e4m3
    state = np.asarray(state, dtype=np.float32)
    J = np.asarray(J, dtype=np.float32)

    # fold the symmetric part into the upper block-triangle
    A = np.triu(J + J.T, 1) + np.diag(np.diag(J))
    A = A.astype(bf16)
    sp8 = np.where(state > 0.5, np.float32(1.0), np.float32(-1.0)).astype(fp8)

    in_maps = []
    placement = []
    for core in range(R * C):
        r, c = divmod(core, C)
        kt_of_pos = []
        for g in range(4):
            grp = [x for x in range(4 * g, 4 * g + 4) if x != 4 * g + r]
            kt_of_pos += grp + [4 * g + r]
        ctile = [4 * j + r for j in range(4)]

        # su: [P, KT, BH]; partition p holds, for position k, the
        # samples of spin row (kt_of_pos[k]*P + p)
        sm = sp8[c * BH:(c + 1) * BH]               # [BH, N]
        su = np.empty((P, KT, BH), dtype=fp8)
        for pos in range(KT):
            kt = kt_of_pos[pos]
            su[:, pos, :] = sm[:, kt * P:(kt + 1) * P].T
        # A tiles in consumption order; zero when above the triangle
        at = np.zeros((P, NTILE, P), dtype=bf16)
        for idx, (s, j) in enumerate(ORDER):
            kt, cj = kt_of_pos[s], ctile[j]
            if kt <= cj:
                at[:, idx, :] = A[kt * P:(kt + 1) * P, cj * P:(cj + 1) * P]
        in_maps.append({
            "su": np.ascontiguousarray(su.reshape(P, KT * BH)),
            "a": np.ascontiguousarray(at.reshape(P, NTILE * P)),
        })
        placement.append((r, c))
    return in_maps, placement


def kernel(state, J):
    from concourse.bass_utils import run_bass_kernel_spmd

    if "nc" not in _cache:
        _cache["nc"] = _build_program()
    nc = _cache["nc"]

    in_maps, placement = _make_in_maps(state, J)
    res = run_bass_kernel_spmd(nc, in_maps, list(range(R * C)))

    out = np.zeros(B, dtype=np.float32)
    for core, (r, c) in enumerate(placement):
        out[c * BH:(c + 1) * BH] += res.results[core]["part"].reshape(BH)
    return out
